# revision 1
# baseline (speedup 1.0000x reference)
"""Lovasz-Softmax loss on Trainium2 (Bass/Tile), 8-core data-parallel over batch.

Math: for each (batch, class c>=1) the Lovasz term equals
    term = sum_{fg pixels i} Phi(e_i) + 1 - G * Phi(1)
where e_i = 1 - p_c(i), G = #fg pixels, Phi(x) = int_0^x dt / (G + b(t)),
and b(t) = #background-valid pixels with p_c > t.  (Derived from the
sorted-cumsum definition via Abel summation; exact.)

Device work per core (1 batch of 262144 pixels x 21 classes):
  softmax -> p256 = 256*p_c; poison fg/invalid elements negative;
  staircase bits S_a = [p256 >= 32a], T_b = [fmod(p256,32) >= 4b]  (a,b in 0..7)
  -> PE matmuls accumulate Csuf[c,a,b] = sum_pix S_a*T_b  (M=64-bin 2D suffix
  histogram per class) plus fg ramp sums R[c,j] = sum_fg clamp(e*16/256 - j, 0, 1).
Host tail (tiny, O(20*64) per batch): 2D-diff -> b(t) at 64 edges -> trapz Phi
-> term per (b,c); include/count logic; final scalar.
"""

import numpy as np
from contextlib import ExitStack

import concourse.bass as bass
import concourse.tile as tile
from concourse import bacc, mybir
from concourse.bass_utils import run_bass_kernel_spmd

F32 = mybir.dt.float32
BF16 = mybir.dt.bfloat16
ALU = mybir.AluOpType
ACTF = mybir.ActivationFunctionType
AXL = mybir.AxisListType

P = 128
C = 21
NCLS = 20
MHI, MLO, MFG = 8, 8, 16
LVL = 32.0   # hi staircase step in p256 units
LOW = 4.0    # lo staircase step
POIS = 2000.0
N_CORES = 8


def _ap(base, extra_off, dims):
    """Custom AP on a tile/dram AP: keep partition dim, replace free dims."""
    return bass.AP(tensor=base.tensor, offset=base.offset + extra_off,
                   ap=[list(base.ap[0])] + [list(d) for d in dims])


def build(ncols=2048, T=64, rep=1):
    assert ncols % T == 0
    NT = ncols // T
    nc = bacc.Bacc("TRN2", target_bir_lowering=False, debug=False,
                   enable_asserts=False, num_devices=N_CORES)
    lg_d = nc.dram_tensor("logits", [P, ncols, C], F32, kind="ExternalInput")
    lab_d = nc.dram_tensor("labels", [P, ncols], F32, kind="ExternalInput")
    cst_d = nc.dram_tensor("consts", [P, 36], F32, kind="ExternalInput")
    o1_d = nc.dram_tensor("out1", [128, 128], F32, kind="ExternalOutput")
    o2_d = nc.dram_tensor("out2a", [32, 32], F32, kind="ExternalOutput")
    o3_d = nc.dram_tensor("out2b", [NCLS, MFG], F32, kind="ExternalOutput")

    with tile.TileContext(nc) as tc, ExitStack() as ctx:
        singles = ctx.enter_context(tc.tile_pool(name="singles", bufs=1))
        pool = ctx.enter_context(tc.tile_pool(name="work", bufs=2))
        psum = ctx.enter_context(
            tc.tile_pool(name="psum", bufs=1, space=bass.MemorySpace.PSUM))

        labs = singles.tile([P, ncols], F32)
        nc.sync.dma_start(labs[:], lab_d.ap())
        cst = singles.tile([P, 36], F32)
        nc.sync.dma_start(cst[:], cst_d.ap())

        ps1 = psum.tile([128, 128], F32)   # 16cls x (8lvl) rows, 16cls x (8lo) cols
        ps2 = psum.tile([32, 32], F32)     # classes 16..19
        ps3 = psum.tile([NCLS, MFG], F32)  # fg ramps

        lg_ap = lg_d.ap()
        labs_ap = labs[:]
        cst_ap = cst[:]

        for rep_i in range(rep):
          for it in range(NT):
            t0 = it * T
            lgt = pool.tile([P, T, C], F32, tag="lg")
            nc.sync.dma_start(
                lgt[:], _ap(lg_ap, t0 * C, [[C, T], [1, C]]))
            ez = pool.tile([P, T, C], F32, tag="ez")
            nc.scalar.activation(ez[:], lgt[:], ACTF.Exp)
            s = pool.tile([P, T], F32, tag="s")
            nc.vector.tensor_reduce(s[:], ez[:], axis=AXL.X, op=ALU.add)
            rc = pool.tile([P, T], F32, tag="rc")
            nc.vector.reciprocal(rc[:], s[:])
            rc256 = pool.tile([P, T], F32, tag="rc256")
            nc.vector.tensor_scalar(rc256[:], rc[:], 256.0, None, ALU.mult)
            poisA = pool.tile([P, T], F32, tag="poisA")
            nc.vector.tensor_scalar(
                poisA[:], _ap(labs_ap, t0, [[1, T]]), 0.0, POIS,
                ALU.is_equal, ALU.mult)

            # fg one-hot over classes 1..20 (f32 + bf16 copy for PE)
            fgm = pool.tile([P, T, NCLS], F32, tag="fgm")
            nc.vector.tensor_tensor(
                fgm[:],
                _ap(labs_ap, t0, [[1, T], [0, NCLS]]),
                _ap(cst_ap, 0, [[0, T], [1, NCLS]]),
                ALU.is_equal)
            fgmh = pool.tile([P, T, NCLS], BF16, tag="fgmh")
            nc.vector.tensor_copy(fgmh[:], fgm[:])

            # pois = fgm*2000 + poisA (broadcast over class)
            pois = pool.tile([P, T, NCLS], F32, tag="pois")
            nc.vector.scalar_tensor_tensor(
                pois[:], fgm[:], POIS,
                _ap(poisA[:], 0, [[1, T], [0, NCLS]]),
                op0=ALU.mult, op1=ALU.add)

            # p256 for classes 1..20, then poisoned pp
            p1 = pool.tile([P, T, NCLS], F32, tag="p1")
            nc.vector.tensor_tensor(
                p1[:],
                _ap(ez[:], 1, [[C, T], [1, NCLS]]),
                _ap(rc256[:], 0, [[1, T], [0, NCLS]]),
                ALU.mult)
            pp = pool.tile([P, T, NCLS], F32, tag="pp")
            nc.vector.tensor_tensor(pp[:], p1[:], pois[:], ALU.subtract)

            # fg pixel value: qfg_raw = sum_c fgm*pp  (= p256_at_label - 2000 on valid)
            tmp = pool.tile([P, T, NCLS], F32, tag="tmp")
            nc.vector.tensor_tensor(tmp[:], fgm[:], pp[:], ALU.mult)
            qfg = pool.tile([P, T], F32, tag="qfg")
            nc.vector.tensor_reduce(qfg[:], tmp[:], axis=AXL.X, op=ALU.add)
            # x16 = e256/16 = (256 - (qfg_raw+2000))/16 = -qfg_raw/16 - 109
            x16 = pool.tile([P, T], F32, tag="x16")
            nc.vector.tensor_scalar(
                x16[:], qfg[:], -1.0 / 16.0, -109.0, ALU.mult, ALU.add)
            u = pool.tile([P, T, MFG], F32, tag="u")
            nc.vector.tensor_tensor(
                u[:],
                _ap(x16[:], 0, [[1, T], [0, MFG]]),
                _ap(cst_ap, 20, [[0, T], [1, MFG]]),
                ALU.subtract)
            rmp = pool.tile([P, T, MFG], BF16, tag="rmp")
            nc.vector.tensor_scalar(rmp[:], u[:], 1.0, 0.0, ALU.min, ALU.max)

            # staircases, split into class groups 0..15 / 16..19 so each
            # per-chunk matmul operand is a single contiguous run
            # (walrus: matmul APs may have only one free dimension).
            # Layout [P, T, lvl, cls] -> per (pixel, t) a flat lvl-major block.
            # qi = floor(pp) as int16 (f32->i16 conversion is round-nearest-even,
            # so subtract 0.5 first); lo residue via bitwise_and.
            I16 = mybir.dt.int16
            qi = pool.tile([P, T, NCLS], I16, tag="qi")
            nc.vector.tensor_scalar(qi[:], pp[:], -0.5, None, ALU.add)
            mi = pool.tile([P, T, NCLS], I16, tag="mi")
            nc.vector.tensor_scalar(mi[:], qi[:], 31, None, ALU.bitwise_and)

            St1 = pool.tile([P, T, MHI, 16], BF16, tag="St1")
            St2 = pool.tile([P, T, MHI, 4], BF16, tag="St2")
            qi_ap = qi[:]
            for a in range(MHI):
                nc.vector.tensor_scalar(
                    _ap(St1[:], a * 16, [[MHI * 16, T], [1, 16]]),
                    _ap(qi_ap, 0, [[NCLS, T], [1, 16]]),
                    int(LVL) * a, None, ALU.is_ge)
                nc.vector.tensor_scalar(
                    _ap(St2[:], a * 4, [[MHI * 4, T], [1, 4]]),
                    _ap(qi_ap, 16, [[NCLS, T], [1, 4]]),
                    int(LVL) * a, None, ALU.is_ge)
            Tt1 = pool.tile([P, T, MLO, 16], BF16, tag="Tt1")
            Tt2 = pool.tile([P, T, MLO, 4], BF16, tag="Tt2")
            mi_ap = mi[:]
            for b in range(MLO):
                nc.vector.tensor_scalar(
                    _ap(Tt1[:], b * 16, [[MLO * 16, T], [1, 16]]),
                    _ap(mi_ap, 0, [[NCLS, T], [1, 16]]),
                    int(LOW) * b, None, ALU.is_ge)
                nc.vector.tensor_scalar(
                    _ap(Tt2[:], b * 4, [[MLO * 4, T], [1, 4]]),
                    _ap(mi_ap, 16, [[NCLS, T], [1, 4]]),
                    int(LOW) * b, None, ALU.is_ge)

            # per-chunk matmuls, accumulating in PSUM
            S1_ap = St1[:]
            S2_ap = St2[:]
            T1_ap = Tt1[:]
            T2_ap = Tt2[:]
            F_ap = fgmh[:]
            R_ap = rmp[:]
            for tt in range(T):
                first = (rep_i == 0 and it == 0 and tt == 0)
                last = (rep_i == rep - 1 and it == NT - 1 and tt == T - 1)
                nc.tensor.matmul(
                    ps1[:],
                    _ap(S1_ap, tt * MHI * 16, [[1, MHI * 16]]),
                    _ap(T1_ap, tt * MLO * 16, [[1, MLO * 16]]),
                    start=first, stop=last)
                nc.tensor.matmul(
                    ps2[:],
                    _ap(S2_ap, tt * MHI * 4, [[1, MHI * 4]]),
                    _ap(T2_ap, tt * MLO * 4, [[1, MLO * 4]]),
                    start=first, stop=last)
                nc.tensor.matmul(
                    ps3[:],
                    _ap(F_ap, tt * NCLS, [[1, NCLS]]),
                    _ap(R_ap, tt * MFG, [[1, MFG]]),
                    start=first, stop=last)

        o1s = singles.tile([128, 128], F32)
        nc.vector.tensor_copy(o1s[:], ps1[:])
        nc.sync.dma_start(o1_d.ap(), o1s[:])
        o2s = singles.tile([32, 32], F32)
        nc.vector.tensor_copy(o2s[:], ps2[:])
        nc.sync.dma_start(o2_d.ap(), o2s[:])
        o3s = singles.tile([NCLS, MFG], F32)
        nc.vector.tensor_copy(o3s[:], ps3[:])
        nc.sync.dma_start(o3_d.ap(), o3s[:])

    nc.compile()
    return nc


def make_consts():
    row = np.concatenate([np.arange(1, 21), np.arange(16)]).astype(np.float32)
    return np.ascontiguousarray(np.broadcast_to(row, (P, 36)))


def host_tail(out1, out2a, out2b, label_counts):
    """Per-batch: decode device accumulators -> 20 Lovasz terms (float64)."""
    M = MHI * MLO
    terms = np.zeros(NCLS)
    for ci in range(NCLS):
        if ci < 16:
            cs = out1[ci::16, ci::16]          # [MHI, MLO], rows lvl-major
        else:
            k = ci - 16
            cs = out2a[k::4, k::4]
        cs = cs.astype(np.float64)              # Csuf[a, b] suffix-suffix counts
        h = cs.copy()
        h[:-1, :] -= cs[1:, :]
        h[:, :-1] -= h[:, 1:]
        hflat = h.reshape(-1)
        bcnt = np.concatenate([np.cumsum(hflat[::-1])[::-1], [0.0]])  # b at M+1 edges
        g = float(label_counts[ci + 1]) if ci + 1 < len(label_counts) else 0.0
        r = 1.0 / np.maximum(g + bcnt, 1.0)
        dt = 1.0 / M
        phi = np.concatenate([[0.0], np.cumsum((r[:-1] + r[1:]) * 0.5 * dt)])
        stride = M // MFG
        dphi = np.diff(phi[::stride])
        fg_term = float((dphi * out2b[ci].astype(np.float64)).sum())
        terms[ci] = fg_term + 1.0 - g * phi[-1]
    return terms


_NC_CACHE = {}


def _get_nc():
    if "nc" not in _NC_CACHE:
        _NC_CACHE["nc"] = build()
    return _NC_CACHE["nc"]


def kernel(logits, labels):
    B, N, Cin = logits.shape
    assert (B, Cin) == (N_CORES, C) and N % P == 0
    ncols = N // P
    logits32 = np.ascontiguousarray(logits, dtype=np.float32).reshape(B, P, ncols, C)
    labf = np.ascontiguousarray(labels.astype(np.float32).reshape(B, P, ncols))
    consts = make_consts()
    nc = _get_nc()
    in_maps = [
        {"logits": logits32[b], "labels": labf[b], "consts": consts}
        for b in range(B)
    ]
    res = run_bass_kernel_spmd(nc, in_maps, core_ids=list(range(N_CORES)))
    _NC_CACHE["last_results"] = res

    lab_int = labels.astype(np.int64)
    total = 0.0
    n_included = 0
    for b in range(B):
        out = res.results[b]
        counts = np.bincount(lab_int[b], minlength=C)
        terms = host_tail(out["out1"], out["out2a"], out["out2b"], counts)
        valid = int(N - counts[0])
        if valid >= 2:
            total += terms.sum()
            n_included += 1
    count = max(n_included * (C - 1), 1)
    return np.float32(total / count)



# revision 16
# speedup vs baseline: 2.1194x; 2.1194x over previous
"""Lovasz-Softmax loss on Trainium2 (Bass/Tile), 8-core data-parallel over batch.

Math: for each (batch, class c>=1) the Lovasz term equals
    term = sum_{fg pixels i} Phi(e_i) + 1 - G * Phi(1)
where e_i = 1 - p_c(i), G = #fg pixels, Phi(x) = int_0^x dt / (G + b(t)),
and b(t) = #background-valid pixels with p_c > t.  (Abel summation; exact.)

Device work per core (1 batch of 262144 pixels x 21 classes), per 128-pixel
group one 81x37 bf16 matmul accumulated in PSUM:
  psq = 256*sqrt(p_c) (negative-poisoned for ignored pixels), qi = floor(psq)
  A = [valid | qi>=64a (a=1..3) x20cls | fg-onehot x20]          (81 cols)
  B = [(qi&63)>=32 x20 | ones | qsq>=32m (m=1..8) | fg ramps j=0..7] (37)
  qsq = psq at the label class; ramps piecewise-linear in xr = 8*sqrt(e).
Bins are sqrt-spaced in p (edges p256 = (32m/16)^2 = 4m^2), which keeps the
trapezoid Phi integral accurate where b(t) varies fastest (small p).
Host tail (tiny): per-class 8-bin bg histogram = all-valid (2D suffix diff)
minus fg suffix counts (U cols); trapezoid Phi on the shared sqrt grid;
terms + include/count logic -> final scalar.  Valid/fg counts come from the
device (ones column), so the host never touches the big arrays.
"""

import math
import numpy as np
from contextlib import ExitStack

import concourse.bass as bass
import concourse.tile as tile
from concourse import bacc, mybir
from concourse.bass_utils import run_bass_kernel_spmd

F32 = mybir.dt.float32
BF16 = mybir.dt.bfloat16
I16 = mybir.dt.int16
ALU = mybir.AluOpType
ACTF = mybir.ActivationFunctionType
AXL = mybir.AxisListType

P = 128
C = 21
NCLS = 20
N_CORES = 8
NA = 81            # A cols: vld(1) + S1..S3 (3*20) + fgm (20)
NB = 37            # B cols: T1 (20) + ones(1) + U (8) + R (8)
COL_ONES = 20
COL_U = 21
COL_R = 29
LN256 = math.log(256.0)
LNHALF = math.log(0.5)


def _ap(base, extra_off, dims):
    """Custom AP on a tile/dram AP: keep partition dim, replace free dims."""
    return bass.AP(tensor=base.tensor, offset=base.offset + extra_off,
                   ap=[list(base.ap[0])] + [list(d) for d in dims])


def build(ncols=2048, T=128):
    assert ncols % T == 0
    NT = ncols // T
    nc = bacc.Bacc("TRN2", target_bir_lowering=False, debug=False,
                   enable_asserts=False, num_devices=N_CORES)
    lg_d = nc.dram_tensor("logits", [P, ncols, C], F32, kind="ExternalInput")
    lab_d = nc.dram_tensor("labels", [P, ncols], BF16, kind="ExternalInput")
    cst_d = nc.dram_tensor("consts", [P, 40], BF16, kind="ExternalInput")
    cstf_d = nc.dram_tensor("cstf", [P, 16], F32, kind="ExternalInput")
    out_d = nc.dram_tensor("out", [NA, NB], F32, kind="ExternalOutput")

    with tile.TileContext(nc) as tc, ExitStack() as ctx:
        singles = ctx.enter_context(tc.tile_pool(name="singles", bufs=1))
        pool = ctx.enter_context(tc.tile_pool(name="work", bufs=2))
        psum = ctx.enter_context(
            tc.tile_pool(name="psum", bufs=1, space=bass.MemorySpace.PSUM))

        labs = singles.tile([P, ncols], BF16)
        nc.sync.dma_start(labs[:], lab_d.ap())
        cst = singles.tile([P, 40], BF16)
        nc.sync.dma_start(cst[:], cst_d.ap())
        cstf = singles.tile([P, 16], F32)
        nc.sync.dma_start(cstf[:], cstf_d.ap())

        ps = psum.tile([NA, NB], F32)
        lg_ap = lg_d.ap()
        labs_ap = labs[:]
        cst_ap = cst[:]
        # per-partition scalar bias APs: [LN256, 256.001, LNHALF, -1..-7]
        bias_ap = lambda k: _ap(cstf[:], k, [[1, 1]])

        for it in range(NT):
            t0 = it * T
            first = it == 0
            last = it == NT - 1

            lgt = pool.tile([P, T, C], F32, tag="lg")
            nc.sync.dma_start(lgt[:], _ap(lg_ap, t0 * C, [[C, T], [1, C]]))

            # Activation engine: ez = e^z, h = e^{z/2}
            ez = pool.tile([P, T, C], BF16, tag="ez")
            nc.scalar.activation(ez[:], lgt[:], ACTF.Exp)
            h = pool.tile([P, T, C], BF16, tag="h")
            nc.scalar.activation(h[:], lgt[:], ACTF.Exp, scale=0.5)

            # s = sum_c e^z (f32, DVE)
            s = pool.tile([P, T], F32, tag="s")
            nc.vector.tensor_reduce(s[:], ez[:], axis=AXL.X, op=ALU.add)

            # Act: g = 256/sqrt(s) via exp(-0.5*ln s + ln 256)
            lns = pool.tile([P, T], F32, tag="lns")
            nc.scalar.activation(lns[:], s[:], ACTF.Ln)
            g = pool.tile([P, T], F32, tag="g")
            nc.scalar.activation(g[:], lns[:], ACTF.Exp, scale=-0.5, bias=bias_ap(0))

            # DVE: m2 = -2 on invalid pixels; g2 = (1+m2)*g duplicated pairs
            m2 = pool.tile([P, T], BF16, tag="m2")
            nc.vector.tensor_scalar(
                m2[:], _ap(labs_ap, t0, [[1, T]]), 0.0, -2.0,
                ALU.is_equal, ALU.mult)
            g2 = pool.tile([P, T, 2], BF16, tag="g2")
            nc.vector.scalar_tensor_tensor(
                _ap(g2[:], 0, [[2, T], [1, 2]]),
                _ap(m2[:], 0, [[1, T], [0, 2]]),
                1.0,
                _ap(g[:], 0, [[1, T], [0, 2]]),
                op0=ALU.add, op1=ALU.mult)

            # psq = h[1:21] * gp  (256*sqrt(p_c)), 2x via pairs
            psq = pool.tile([P, T, NCLS], BF16, tag="psq")
            nc.vector.tensor_tensor(
                _ap(psq[:], 0, [[NCLS, T], [2, 10], [1, 2]]),
                _ap(h[:], 1, [[C, T], [2, 10], [1, 2]]),
                _ap(g2[:], 0, [[2, T], [0, 10], [1, 2]]),
                ALU.mult)
            qi = pool.tile([P, T, NCLS], I16, tag="qi")
            nc.vector.tensor_scalar(qi[:], psq[:], -0.5, None, ALU.add)

            A = pool.tile([P, T, NA], BF16, tag="A")
            B = pool.tile([P, T, NB], BF16, tag="B")
            qi_ap = qi[:]

            # A col 0: valid
            nc.vector.tensor_scalar(
                _ap(A[:], 0, [[NA, T], [1, 1]]),
                _ap(labs_ap, t0, [[1, T], [0, 1]]),
                0.5, None, ALU.is_ge)
            # A cols 1..60: S_a = qi >= 64a (DVE 4x)
            for a in (1, 2, 3):
                nc.vector.tensor_scalar(
                    _ap(A[:], 1 + (a - 1) * NCLS, [[NA, T], [1, NCLS]]),
                    qi_ap, 64 * a, None, ALU.is_ge)
            # A cols 61..80: fg one-hot (DVE 2x via pairs)
            lab2 = pool.tile([P, T, 2], BF16, tag="lab2")
            nc.vector.tensor_copy(
                lab2[:], _ap(labs_ap, t0, [[1, T], [0, 2]]))
            nc.vector.tensor_tensor(
                _ap(A[:], 1 + 3 * NCLS, [[NA, T], [2, 10], [1, 2]]),
                _ap(lab2[:], 0, [[2, T], [0, 10], [1, 2]]),
                _ap(cst_ap, 0, [[0, T], [2, 10], [1, 2]]),
                ALU.is_equal)

            # B cols 0..19: T1 = (qi & 32) >= 1 (DVE 4x, two ops)
            mi = pool.tile([P, T, NCLS], I16, tag="mi")
            nc.vector.tensor_scalar(mi[:], qi_ap, 32, None, ALU.bitwise_and)
            nc.vector.tensor_scalar(
                _ap(B[:], 0, [[NB, T], [1, NCLS]]),
                mi[:], 1, None, ALU.is_ge)
            # B col 20: ones (Pool memset)
            nc.gpsimd.memset(_ap(B[:], COL_ONES, [[NB, T], [1, 1]]), 1.0)

            # qsq = psq at label (fg) via one-hot mult + reduce
            tmp = pool.tile([P, T, NCLS], BF16, tag="tmp")
            nc.vector.tensor_tensor(
                tmp[:],
                _ap(A[:], 1 + 3 * NCLS, [[NA, T], [1, NCLS]]),
                psq[:], ALU.mult)
            qsq = pool.tile([P, T], F32, tag="qsq")
            nc.vector.tensor_reduce(qsq[:], tmp[:], axis=AXL.X, op=ALU.add)
            # clamped bf16 pairs of qsq
            qsq2 = pool.tile([P, T, 2], BF16, tag="qsq2")
            nc.vector.tensor_scalar(
                _ap(qsq2[:], 0, [[2, T], [1, 2]]),
                _ap(qsq[:], 0, [[1, T], [0, 2]]),
                255.0, None, ALU.min)
            # B cols 21..28: U_m = qsq >= 32m (DVE 2x via pairs)
            nc.vector.tensor_tensor(
                _ap(B[:], COL_U, [[NB, T], [2, 4], [1, 2]]),
                _ap(qsq2[:], 0, [[2, T], [0, 4], [1, 2]]),
                _ap(cst_ap, NCLS, [[0, T], [2, 4], [1, 2]]),
                ALU.is_ge)

            # fg ramp coord xr = 0.5*sqrt(256.001 - (qsq/16)^2) in [0,8]
            qfg = pool.tile([P, T], F32, tag="qfg")
            nc.scalar.activation(qfg[:], _ap(qsq2[:], 0, [[2, T], [1, 1]]),
                                 ACTF.Square, scale=1.0 / 16.0)
            lx = pool.tile([P, T], F32, tag="lx")
            nc.scalar.activation(lx[:], qfg[:], ACTF.Ln, scale=-1.0, bias=bias_ap(1))
            xr = pool.tile([P, T], F32, tag="xr")
            nc.scalar.activation(xr[:], lx[:], ACTF.Exp, scale=0.5, bias=bias_ap(2))
            # B cols 29..36: ramps relu(xr - j) via Act, then clamp min 1 (DVE)
            for j in range(8):
                nc.scalar.activation(
                    _ap(B[:], COL_R + j, [[NB, T], [1, 1]]),
                    _ap(xr[:], 0, [[1, T], [0, 1]]),
                    ACTF.Relu, bias=(0.0 if j == 0 else bias_ap(2 + j)))
            nc.vector.tensor_scalar(
                _ap(B[:], COL_R, [[NB, T], [1, 8]]),
                _ap(B[:], COL_R, [[NB, T], [1, 8]]),
                1.0, None, ALU.min)

            A_ap = A[:]
            B_ap = B[:]
            for tt_ in range(T):
                nc.tensor.matmul(
                    ps[:],
                    _ap(A_ap, tt_ * NA, [[1, NA]]),
                    _ap(B_ap, tt_ * NB, [[1, NB]]),
                    start=(first and tt_ == 0),
                    stop=(last and tt_ == T - 1))

        outs = singles.tile([NA, NB], F32)
        nc.vector.tensor_copy(outs[:], ps[:])
        nc.sync.dma_start(out_d.ap(), outs[:])

    nc.compile()
    return nc


def make_consts():
    row = np.zeros(40, np.float32)
    row[0:20] = np.arange(1, 21)           # fgm class ids
    row[20:28] = 32.0 * np.arange(1, 9)    # U edges (qsq units)
    row[28:36] = np.arange(8)              # ramp nodes (xr units)
    import ml_dtypes
    cb = np.ascontiguousarray(
        np.broadcast_to(row.astype(ml_dtypes.bfloat16), (P, 40)))
    rf = np.zeros(16, np.float32)
    rf[0] = LN256
    rf[1] = 256.001
    rf[2] = LNHALF
    rf[3:10] = -np.arange(1, 8)
    cf = np.ascontiguousarray(np.broadcast_to(rf, (P, 16)))
    return cb, cf


def host_tail(out):
    """out[NA, NB] f32 -> (terms[20], V) in float64."""
    o = out.astype(np.float64)
    vld_row = o[0]
    S_rows = o[1:61].reshape(3, NCLS, NB)
    F_rows = o[61:81]
    V = vld_row[COL_ONES]
    G = F_rows[:, COL_ONES]

    # 2D suffix counts -> per-bin -> bg suffix, vectorized over classes
    Cs = np.zeros((NCLS, 5, 2))
    Cs[:, 0, 0] = V
    Cs[:, 0, 1] = vld_row[:NCLS]
    for a in range(1, 4):
        Cs[:, a, 0] = S_rows[a - 1, :, COL_ONES]
        Cs[:, a, 1] = S_rows[a - 1, np.arange(NCLS), np.arange(NCLS)]
    D = Cs[:, :4, :] - Cs[:, 1:5, :]
    hcell = D.copy()
    hcell[:, :, 0] -= hcell[:, :, 1]
    hflat = hcell.reshape(NCLS, 8)
    bcnt = np.concatenate(
        [np.cumsum(hflat[:, ::-1], axis=1)[:, ::-1],
         np.zeros((NCLS, 1))], axis=1)                  # (20, 9) suffix m=0..8
    F = np.concatenate([G[:, None], F_rows[:, COL_U:COL_U + 8]], axis=1)
    bg = np.maximum(bcnt - F, 0.0)
    r = 1.0 / np.maximum(G[:, None] + bg, 1.0)
    edges = (np.arange(9) ** 2) / 64.0
    dphi = (r[:, :-1] + r[:, 1:]) * 0.5 * np.diff(edges)[None, :]
    phi1 = dphi.sum(axis=1)
    fg_term = (dphi * F_rows[:, COL_R:COL_R + 8]).sum(axis=1)
    terms = fg_term + 1.0 - G * phi1
    return terms, V


_NC_CACHE = {}


def _get_nc():
    if "nc" not in _NC_CACHE:
        _NC_CACHE["nc"] = build()
    return _NC_CACHE["nc"]


def _get_exec():
    """Cached jitted SPMD executor taking global (concat) arrays directly,
    avoiding run_bass_via_pjrt's per-call 176MB concat + jit retrace."""
    if "exec" in _NC_CACHE:
        return _NC_CACHE["exec"]
    import jax
    from jax.sharding import Mesh, PartitionSpec
    from concourse.bass2jax import (
        _bass_exec_p, install_neuronx_cc_hook, partition_id_tensor)

    nc = _get_nc()
    install_neuronx_cc_hook()
    partition_name = (nc.partition_id_tensor.name
                      if nc.partition_id_tensor else None)

    in_names, out_names, out_avals, zero_outs = [], [], [], []
    for alloc in nc.m.functions[0].allocations:
        if not isinstance(alloc, mybir.MemoryLocationSet):
            continue
        name = alloc.memorylocations[0].name
        if alloc.kind == "ExternalInput":
            if name != partition_name:
                in_names.append(name)
        elif alloc.kind == "ExternalOutput":
            shape = tuple(alloc.tensor_shape)
            dtype = mybir.dt.np(alloc.dtype)
            out_names.append(name)
            out_avals.append(jax.core.ShapedArray(shape, dtype))
            zero_outs.append(
                np.zeros((N_CORES * shape[0], *shape[1:]), dtype))
    n_params = len(in_names)
    all_names = in_names + out_names
    if partition_name is not None:
        all_names = all_names + [partition_name]
    donate = tuple(range(n_params, n_params + len(out_names)))

    def _body(*args):
        operands = list(args)
        if partition_name is not None:
            operands.append(partition_id_tensor())
        outs = _bass_exec_p.bind(
            *operands,
            out_avals=tuple(out_avals),
            in_names=tuple(all_names),
            out_names=tuple(out_names),
            lowering_input_output_aliases=(),
            sim_require_finite=True,
            sim_require_nnan=True,
            nc=nc,
        )
        return tuple(outs)

    try:
        from jax import shard_map
    except ImportError:
        from jax.experimental.shard_map import shard_map
    devices = jax.devices()[:N_CORES]
    assert len(devices) == N_CORES, (
        f"need {N_CORES} devices, have {len(jax.devices())}")
    mesh = Mesh(np.asarray(devices), ("core",))
    nio = n_params + len(out_names)
    sharded = jax.jit(
        shard_map(_body, mesh=mesh,
                  in_specs=(PartitionSpec("core"),) * nio,
                  out_specs=(PartitionSpec("core"),) * len(out_names),
                  check_vma=False),
        donate_argnums=donate, keep_unused=True)
    _NC_CACHE["exec"] = (sharded, in_names, out_names, out_avals, zero_outs)
    return _NC_CACHE["exec"]


def kernel(logits, labels):
    import os
    import ml_dtypes
    B, N, Cin = logits.shape
    assert (B, Cin) == (N_CORES, C) and N % P == 0
    ncols = N // P

    logits_g = np.ascontiguousarray(logits, dtype=np.float32) \
        .reshape(B * P, ncols, C)
    labf_g = labels.reshape(B * P, ncols).astype(ml_dtypes.bfloat16)
    consts, constsf = make_consts()
    consts_g = np.ascontiguousarray(
        np.broadcast_to(consts[None, 0], (B * P, consts.shape[1])))
    constsf_g = np.ascontiguousarray(
        np.broadcast_to(constsf[None, 0], (B * P, constsf.shape[1])))

    if os.environ.get("BASS_TRACE"):
        # slow traced path through run_bass_kernel_spmd for profiling
        nc = _get_nc()
        in_maps = [
            {"logits": logits_g[b * P:(b + 1) * P],
             "labels": labf_g[b * P:(b + 1) * P],
             "consts": consts, "cstf": constsf}
            for b in range(B)
        ]
        res = run_bass_kernel_spmd(nc, in_maps, core_ids=list(range(N_CORES)))
        _NC_CACHE["last_results"] = res
        outs = [res.results[b]["out"] for b in range(B)]
    else:
        sharded, in_names, out_names, out_avals, zero_outs = _get_exec()
        arrs = {"logits": logits_g, "labels": labf_g,
                "consts": consts_g, "cstf": constsf_g}
        out_arrs = sharded(*[arrs[n] for n in in_names],
                           *[z.copy() for z in zero_outs])
        og = np.asarray(out_arrs[out_names.index("out")])
        outs = [og.reshape(N_CORES, NA, NB)[b] for b in range(B)]

    total = 0.0
    n_included = 0
    for b in range(B):
        terms, V = host_tail(outs[b])
        if V >= 2:
            total += terms.sum()
            n_included += 1
    count = max(n_included * (C - 1), 1)
    return np.float32(total / count)


# revision 22
# speedup vs baseline: 2.7560x; 1.3004x over previous
"""Lovasz-Softmax loss on Trainium2 (Bass/Tile), 8-core data-parallel over batch.

Math: for each (batch, class c>=1) the Lovasz term equals
    term = sum_{fg pixels i} Phi(e_i) + 1 - G * Phi(1)
where e_i = 1 - p_c(i), G = #fg pixels, Phi(x) = int_0^x dt / (G + b(t)),
and b(t) = #background-valid pixels with p_c > t.  (Abel summation; exact.)

Device work per core (1 batch of 262144 pixels x 21 classes), per 128-pixel
group one 81x37 bf16 matmul accumulated in PSUM:
  psq = 256*sqrt(p_c) (negative-poisoned for ignored pixels), qi = floor(psq)
  A = [valid | qi>=64a (a=1..3) x20cls | fg-onehot x20]          (81 cols)
  B = [(qi&63)>=32 x20 | ones | qsq>=32m (m=1..8) | fg ramps j=0..7] (37)
  qsq = psq at the label class; ramps piecewise-linear in xr = 8*sqrt(e).
Bins are sqrt-spaced in p (edges p256 = (32m/16)^2 = 4m^2), which keeps the
trapezoid Phi integral accurate where b(t) varies fastest (small p).
Host tail (tiny): per-class 8-bin bg histogram = all-valid (2D suffix diff)
minus fg suffix counts (U cols); trapezoid Phi on the shared sqrt grid;
terms + include/count logic -> final scalar.  Valid/fg counts come from the
device (ones column), so the host never touches the big arrays.
"""

import math
import numpy as np
from contextlib import ExitStack

import concourse.bass as bass
import concourse.tile as tile
from concourse import bacc, mybir
from concourse.bass_utils import run_bass_kernel_spmd

F32 = mybir.dt.float32
BF16 = mybir.dt.bfloat16
I16 = mybir.dt.int16
ALU = mybir.AluOpType
ACTF = mybir.ActivationFunctionType
AXL = mybir.AxisListType

P = 128
C = 21
NCLS = 20
N_CORES = 8
NA = 81            # A cols: vld(1) + S1..S3 (3*20) + fgm (20)
NB = 37            # B cols: T1 (20) + ones(1) + U (8) + R (8)
COL_ONES = 20
COL_U = 21
COL_R = 29
LN256 = math.log(256.0)
LNHALF = math.log(0.5)


def _ap(base, extra_off, dims):
    """Custom AP on a tile/dram AP: keep partition dim, replace free dims."""
    return bass.AP(tensor=base.tensor, offset=base.offset + extra_off,
                   ap=[list(base.ap[0])] + [list(d) for d in dims])


def _pin_act_tables():
    """Make the act-table-load pass assign every activation to the one set
    containing all funcs we use (natural_log_exp_and_others), instead of
    thrashing between the exp and ln sets (stateless first-match).  Set ids
    stay aligned with act_info.json because only set CONTENTS are filtered,
    never the order.  Returns a restore function."""
    import concourse.hw_specs as hw_specs
    orig = hw_specs.get_activation_tables
    mine = {ACTF.Exp, ACTF.Ln, ACTF.Square, ACTF.Relu}

    def patched(arch):
        t = orig(arch)
        out = {}
        for name, funcs in t.items():
            if name == "natural_log_exp_and_others":
                out[name] = funcs
            else:
                out[name] = funcs - mine
        return out

    hw_specs.get_activation_tables = patched
    import concourse.bacc as bacc_mod
    had = getattr(bacc_mod, "get_activation_tables", None)
    if had is not None:
        bacc_mod.get_activation_tables = patched

    def restore():
        hw_specs.get_activation_tables = orig
        if had is not None:
            bacc_mod.get_activation_tables = orig
    return restore


def build(ncols=2048, T=128):
    assert ncols % T == 0
    NT = ncols // T
    restore_tables = _pin_act_tables()
    nc = bacc.Bacc("TRN2", target_bir_lowering=False, debug=False,
                   enable_asserts=False, num_devices=N_CORES)
    lg_d = nc.dram_tensor("logits", [P, ncols, C], F32, kind="ExternalInput")
    lab_d = nc.dram_tensor("labels", [P, ncols], BF16, kind="ExternalInput")
    cst_d = nc.dram_tensor("consts", [P, 40], BF16, kind="ExternalInput")
    cstf_d = nc.dram_tensor("cstf", [P, 16], F32, kind="ExternalInput")
    out_d = nc.dram_tensor("out", [NA, NB], F32, kind="ExternalOutput")

    with tile.TileContext(nc) as tc, ExitStack() as ctx:
        singles = ctx.enter_context(tc.tile_pool(name="singles", bufs=1))
        pool = ctx.enter_context(tc.tile_pool(name="work", bufs=2))
        abpool = ctx.enter_context(tc.tile_pool(name="ab", bufs=3))
        psum = ctx.enter_context(
            tc.tile_pool(name="psum", bufs=1, space=bass.MemorySpace.PSUM))

        labs = singles.tile([P, ncols], BF16)
        nc.sync.dma_start(labs[:], lab_d.ap())
        cst = singles.tile([P, 40], BF16)
        nc.sync.dma_start(cst[:], cst_d.ap())
        cstf = singles.tile([P, 16], F32)
        nc.sync.dma_start(cstf[:], cstf_d.ap())

        ps = psum.tile([NA, NB], F32)
        lg_ap = lg_d.ap()
        labs_ap = labs[:]
        cst_ap = cst[:]
        # per-partition scalar bias APs: [LN256, 256.001, LNHALF, -1..-7]
        bias_ap = lambda k: _ap(cstf[:], k, [[1, 1]])

        for it in range(NT):
            t0 = it * T
            first = it == 0
            last = it == NT - 1

            lgt = pool.tile([P, T, C], F32, tag="lg")
            nc.sync.dma_start(lgt[:], _ap(lg_ap, t0 * C, [[C, T], [1, C]]))

            # Activation engine: ez = e^z, h = e^{z/2}
            ez = pool.tile([P, T, C], BF16, tag="ez")
            nc.scalar.activation(ez[:], lgt[:], ACTF.Exp)
            h = pool.tile([P, T, C], BF16, tag="h")
            nc.scalar.activation(h[:], lgt[:], ACTF.Exp, scale=0.5)

            # s = sum_c e^z (f32): in-place halving tree on ez, then reduce
            ez_ap = ez[:]
            nc.vector.tensor_tensor(
                _ap(ez_ap, 0, [[C, T], [1, 1]]),
                _ap(ez_ap, 0, [[C, T], [1, 1]]),
                _ap(ez_ap, 20, [[C, T], [1, 1]]), ALU.add)
            nc.vector.tensor_tensor(
                _ap(ez_ap, 0, [[C, T], [1, 10]]),
                _ap(ez_ap, 0, [[C, T], [1, 10]]),
                _ap(ez_ap, 10, [[C, T], [1, 10]]), ALU.add)
            nc.vector.tensor_tensor(
                _ap(ez_ap, 0, [[C, T], [1, 5]]),
                _ap(ez_ap, 0, [[C, T], [1, 5]]),
                _ap(ez_ap, 5, [[C, T], [1, 5]]), ALU.add)
            s = pool.tile([P, T], F32, tag="s")
            nc.vector.tensor_reduce(
                s[:], _ap(ez_ap, 0, [[C, T], [1, 5]]), axis=AXL.X, op=ALU.add)

            # Act: g = 256/sqrt(s) via exp(-0.5*ln s + ln 256)
            lns = pool.tile([P, T], F32, tag="lns")
            nc.scalar.activation(lns[:], s[:], ACTF.Ln)
            g = pool.tile([P, T], F32, tag="g")
            nc.scalar.activation(g[:], lns[:], ACTF.Exp, scale=-0.5, bias=bias_ap(0))

            # DVE: m2 = -2 on invalid pixels; g2 = (1+m2)*g duplicated pairs
            m2 = pool.tile([P, T], BF16, tag="m2")
            nc.vector.tensor_scalar(
                m2[:], _ap(labs_ap, t0, [[1, T]]), 0.0, -2.0,
                ALU.is_equal, ALU.mult)
            g2 = pool.tile([P, T, 2], BF16, tag="g2")
            nc.vector.scalar_tensor_tensor(
                _ap(g2[:], 0, [[2, T], [1, 2]]),
                _ap(m2[:], 0, [[1, T], [0, 2]]),
                1.0,
                _ap(g[:], 0, [[1, T], [0, 2]]),
                op0=ALU.add, op1=ALU.mult)

            # psq = h[1:21] * gp  (256*sqrt(p_c)), 2x via pairs
            psq = pool.tile([P, T, NCLS], BF16, tag="psq")
            nc.vector.tensor_tensor(
                _ap(psq[:], 0, [[NCLS, T], [2, 10], [1, 2]]),
                _ap(h[:], 1, [[C, T], [2, 10], [1, 2]]),
                _ap(g2[:], 0, [[2, T], [0, 10], [1, 2]]),
                ALU.mult)
            qi = pool.tile([P, T, NCLS], I16, tag="qi")
            nc.vector.tensor_scalar(qi[:], psq[:], -0.5, None, ALU.add)

            A = abpool.tile([P, T, NA], BF16, tag="A")
            B = abpool.tile([P, T, NB], BF16, tag="B")
            qi_ap = qi[:]

            # A col 0: valid
            nc.vector.tensor_scalar(
                _ap(A[:], 0, [[NA, T], [1, 1]]),
                _ap(labs_ap, t0, [[1, T], [0, 1]]),
                0.5, None, ALU.is_ge)
            # A cols 1..60: S_a = qi >= 64a (DVE 4x)
            for a in (1, 2, 3):
                nc.vector.tensor_scalar(
                    _ap(A[:], 1 + (a - 1) * NCLS, [[NA, T], [1, NCLS]]),
                    qi_ap, 64 * a, None, ALU.is_ge)
            # A cols 61..80: fg one-hot (DVE 2x via pairs)
            lab2 = pool.tile([P, T, 2], BF16, tag="lab2")
            nc.vector.tensor_copy(
                lab2[:], _ap(labs_ap, t0, [[1, T], [0, 2]]))
            nc.vector.tensor_tensor(
                _ap(A[:], 1 + 3 * NCLS, [[NA, T], [2, 10], [1, 2]]),
                _ap(lab2[:], 0, [[2, T], [0, 10], [1, 2]]),
                _ap(cst_ap, 0, [[0, T], [2, 10], [1, 2]]),
                ALU.is_equal)

            # B cols 0..19: T1 = (qi & 32) >= 1 (DVE 4x, two ops)
            mi = pool.tile([P, T, NCLS], I16, tag="mi")
            nc.vector.tensor_scalar(mi[:], qi_ap, 32, None, ALU.bitwise_and)
            nc.vector.tensor_scalar(
                _ap(B[:], 0, [[NB, T], [1, NCLS]]),
                mi[:], 1, None, ALU.is_ge)
            # B col 20: ones (Pool memset)
            nc.gpsimd.memset(_ap(B[:], COL_ONES, [[NB, T], [1, 1]]), 1.0)

            # qsq = psq at label (fg) via one-hot mult + halving tree reduce
            tmp = pool.tile([P, T, NCLS], BF16, tag="tmp")
            nc.vector.tensor_tensor(
                tmp[:],
                _ap(A[:], 1 + 3 * NCLS, [[NA, T], [1, NCLS]]),
                psq[:], ALU.mult)
            tmp_ap = tmp[:]
            nc.vector.tensor_tensor(
                _ap(tmp_ap, 0, [[NCLS, T], [1, 10]]),
                _ap(tmp_ap, 0, [[NCLS, T], [1, 10]]),
                _ap(tmp_ap, 10, [[NCLS, T], [1, 10]]), ALU.add)
            nc.vector.tensor_tensor(
                _ap(tmp_ap, 0, [[NCLS, T], [1, 5]]),
                _ap(tmp_ap, 0, [[NCLS, T], [1, 5]]),
                _ap(tmp_ap, 5, [[NCLS, T], [1, 5]]), ALU.add)
            qsq = pool.tile([P, T], F32, tag="qsq")
            nc.vector.tensor_reduce(
                qsq[:], _ap(tmp_ap, 0, [[NCLS, T], [1, 5]]),
                axis=AXL.X, op=ALU.add)
            # clamped bf16 pairs of qsq
            qsq2 = pool.tile([P, T, 2], BF16, tag="qsq2")
            nc.vector.tensor_scalar(
                _ap(qsq2[:], 0, [[2, T], [1, 2]]),
                _ap(qsq[:], 0, [[1, T], [0, 2]]),
                255.0, None, ALU.min)
            # B cols 21..28: U_m = qsq >= 32m (DVE 2x via pairs)
            nc.vector.tensor_tensor(
                _ap(B[:], COL_U, [[NB, T], [2, 4], [1, 2]]),
                _ap(qsq2[:], 0, [[2, T], [0, 4], [1, 2]]),
                _ap(cst_ap, NCLS, [[0, T], [2, 4], [1, 2]]),
                ALU.is_ge)

            # fg ramp coord xr = 0.5*sqrt(256.001 - (qsq/16)^2) in [0,8]
            qfg = pool.tile([P, T], F32, tag="qfg")
            nc.scalar.activation(qfg[:], _ap(qsq2[:], 0, [[2, T], [1, 1]]),
                                 ACTF.Square, scale=1.0 / 16.0)
            lx = pool.tile([P, T], F32, tag="lx")
            nc.scalar.activation(lx[:], qfg[:], ACTF.Ln, scale=-1.0, bias=bias_ap(1))
            xr = pool.tile([P, T], BF16, tag="xr")
            nc.scalar.activation(xr[:], lx[:], ACTF.Exp, scale=0.5, bias=bias_ap(2))
            # B cols 29..36: ramps clip(xr - j, 0, 1) (DVE: pairs sub + clamp)
            xr2 = pool.tile([P, T, 2], BF16, tag="xr2")
            nc.vector.tensor_copy(xr2[:], _ap(xr[:], 0, [[1, T], [0, 2]]))
            u = pool.tile([P, T, 8], BF16, tag="u")
            nc.vector.tensor_tensor(
                _ap(u[:], 0, [[8, T], [2, 4], [1, 2]]),
                _ap(xr2[:], 0, [[2, T], [0, 4], [1, 2]]),
                _ap(cst_ap, NCLS + 8, [[0, T], [2, 4], [1, 2]]),
                ALU.subtract)
            nc.vector.tensor_scalar(
                _ap(B[:], COL_R, [[NB, T], [1, 8]]),
                u[:], 1.0, 0.0, ALU.min, ALU.max)

            A_ap = A[:]
            B_ap = B[:]
            for tt_ in range(T):
                nc.tensor.matmul(
                    ps[:],
                    _ap(A_ap, tt_ * NA, [[1, NA]]),
                    _ap(B_ap, tt_ * NB, [[1, NB]]),
                    start=(first and tt_ == 0),
                    stop=(last and tt_ == T - 1))

        outs = singles.tile([NA, NB], F32)
        nc.vector.tensor_copy(outs[:], ps[:])
        nc.sync.dma_start(out_d.ap(), outs[:])

    nc.compile()
    restore_tables()
    return nc


def make_consts():
    row = np.zeros(40, np.float32)
    row[0:20] = np.arange(1, 21)           # fgm class ids
    row[20:28] = 32.0 * np.arange(1, 9)    # U edges (qsq units)
    row[28:36] = np.arange(8)              # ramp nodes (xr units)
    import ml_dtypes
    cb = np.ascontiguousarray(
        np.broadcast_to(row.astype(ml_dtypes.bfloat16), (P, 40)))
    rf = np.zeros(16, np.float32)
    rf[0] = LN256
    rf[1] = 256.001
    rf[2] = LNHALF
    rf[3:10] = -np.arange(1, 8)
    cf = np.ascontiguousarray(np.broadcast_to(rf, (P, 16)))
    return cb, cf


def host_tail(out):
    """out[NA, NB] f32 -> (terms[20], V) in float64."""
    o = out.astype(np.float64)
    vld_row = o[0]
    S_rows = o[1:61].reshape(3, NCLS, NB)
    F_rows = o[61:81]
    V = vld_row[COL_ONES]
    G = F_rows[:, COL_ONES]

    # 2D suffix counts -> per-bin -> bg suffix, vectorized over classes
    Cs = np.zeros((NCLS, 5, 2))
    Cs[:, 0, 0] = V
    Cs[:, 0, 1] = vld_row[:NCLS]
    for a in range(1, 4):
        Cs[:, a, 0] = S_rows[a - 1, :, COL_ONES]
        Cs[:, a, 1] = S_rows[a - 1, np.arange(NCLS), np.arange(NCLS)]
    D = Cs[:, :4, :] - Cs[:, 1:5, :]
    hcell = D.copy()
    hcell[:, :, 0] -= hcell[:, :, 1]
    hflat = hcell.reshape(NCLS, 8)
    bcnt = np.concatenate(
        [np.cumsum(hflat[:, ::-1], axis=1)[:, ::-1],
         np.zeros((NCLS, 1))], axis=1)                  # (20, 9) suffix m=0..8
    F = np.concatenate([G[:, None], F_rows[:, COL_U:COL_U + 8]], axis=1)
    bg = np.maximum(bcnt - F, 0.0)
    r = 1.0 / np.maximum(G[:, None] + bg, 1.0)
    edges = (np.arange(9) ** 2) / 64.0
    dphi = (r[:, :-1] + r[:, 1:]) * 0.5 * np.diff(edges)[None, :]
    phi1 = dphi.sum(axis=1)
    fg_term = (dphi * F_rows[:, COL_R:COL_R + 8]).sum(axis=1)
    terms = fg_term + 1.0 - G * phi1
    return terms, V


_NC_CACHE = {}


def _get_nc():
    if "nc" not in _NC_CACHE:
        _NC_CACHE["nc"] = build()
    return _NC_CACHE["nc"]


def _get_exec():
    """Cached jitted SPMD executor taking global (concat) arrays directly,
    avoiding run_bass_via_pjrt's per-call 176MB concat + jit retrace."""
    if "exec" in _NC_CACHE:
        return _NC_CACHE["exec"]
    import jax
    from jax.sharding import Mesh, PartitionSpec
    from concourse.bass2jax import (
        _bass_exec_p, install_neuronx_cc_hook, partition_id_tensor)

    nc = _get_nc()
    install_neuronx_cc_hook()
    partition_name = (nc.partition_id_tensor.name
                      if nc.partition_id_tensor else None)

    in_names, out_names, out_avals, zero_outs = [], [], [], []
    for alloc in nc.m.functions[0].allocations:
        if not isinstance(alloc, mybir.MemoryLocationSet):
            continue
        name = alloc.memorylocations[0].name
        if alloc.kind == "ExternalInput":
            if name != partition_name:
                in_names.append(name)
        elif alloc.kind == "ExternalOutput":
            shape = tuple(alloc.tensor_shape)
            dtype = mybir.dt.np(alloc.dtype)
            out_names.append(name)
            out_avals.append(jax.core.ShapedArray(shape, dtype))
            zero_outs.append(
                np.zeros((N_CORES * shape[0], *shape[1:]), dtype))
    n_params = len(in_names)
    all_names = in_names + out_names
    if partition_name is not None:
        all_names = all_names + [partition_name]
    donate = tuple(range(n_params, n_params + len(out_names)))

    def _body(*args):
        operands = list(args)
        if partition_name is not None:
            operands.append(partition_id_tensor())
        outs = _bass_exec_p.bind(
            *operands,
            out_avals=tuple(out_avals),
            in_names=tuple(all_names),
            out_names=tuple(out_names),
            lowering_input_output_aliases=(),
            sim_require_finite=True,
            sim_require_nnan=True,
            nc=nc,
        )
        return tuple(outs)

    try:
        from jax import shard_map
    except ImportError:
        from jax.experimental.shard_map import shard_map
    devices = jax.devices()[:N_CORES]
    assert len(devices) == N_CORES, (
        f"need {N_CORES} devices, have {len(jax.devices())}")
    mesh = Mesh(np.asarray(devices), ("core",))
    nio = n_params + len(out_names)
    sharded = jax.jit(
        shard_map(_body, mesh=mesh,
                  in_specs=(PartitionSpec("core"),) * nio,
                  out_specs=(PartitionSpec("core"),) * len(out_names),
                  check_vma=False),
        donate_argnums=donate, keep_unused=True)
    _NC_CACHE["exec"] = (sharded, in_names, out_names, out_avals, zero_outs)
    return _NC_CACHE["exec"]


def kernel(logits, labels):
    import os
    import ml_dtypes
    B, N, Cin = logits.shape
    assert (B, Cin) == (N_CORES, C) and N % P == 0
    ncols = N // P

    logits_g = np.ascontiguousarray(logits, dtype=np.float32) \
        .reshape(B * P, ncols, C)
    labf_g = labels.reshape(B * P, ncols).astype(ml_dtypes.bfloat16)
    consts, constsf = make_consts()
    consts_g = np.ascontiguousarray(
        np.broadcast_to(consts[None, 0], (B * P, consts.shape[1])))
    constsf_g = np.ascontiguousarray(
        np.broadcast_to(constsf[None, 0], (B * P, constsf.shape[1])))

    if os.environ.get("BASS_TRACE"):
        # slow traced path through run_bass_kernel_spmd for profiling
        nc = _get_nc()
        in_maps = [
            {"logits": logits_g[b * P:(b + 1) * P],
             "labels": labf_g[b * P:(b + 1) * P],
             "consts": consts, "cstf": constsf}
            for b in range(B)
        ]
        res = run_bass_kernel_spmd(nc, in_maps, core_ids=list(range(N_CORES)))
        _NC_CACHE["last_results"] = res
        outs = [res.results[b]["out"] for b in range(B)]
    else:
        sharded, in_names, out_names, out_avals, zero_outs = _get_exec()
        arrs = {"logits": logits_g, "labels": labf_g,
                "consts": consts_g, "cstf": constsf_g}
        out_arrs = sharded(*[arrs[n] for n in in_names],
                           *[z.copy() for z in zero_outs])
        og = np.asarray(out_arrs[out_names.index("out")])
        outs = [og.reshape(N_CORES, NA, NB)[b] for b in range(B)]

    total = 0.0
    n_included = 0
    for b in range(B):
        terms, V = host_tail(outs[b])
        if V >= 2:
            total += terms.sum()
            n_included += 1
    count = max(n_included * (C - 1), 1)
    return np.float32(total / count)


# revision 24
# speedup vs baseline: 3.0687x; 1.1134x over previous
"""Lovasz-Softmax loss on Trainium2 (Bass/Tile), 8-core data-parallel over batch.

Math: for each (batch, class c>=1) the Lovasz term equals
    term = sum_{fg pixels i} Phi(e_i) + 1 - G * Phi(1)
where e_i = 1 - p_c(i), G = #fg pixels, Phi(x) = int_0^x dt / (G + b(t)),
and b(t) = #background-valid pixels with p_c > t.  (Abel summation; exact.)

Device work per core (1 batch of 262144 pixels x 21 classes), per 128-pixel
group one 41x33 bf16 matmul accumulated in PSUM:
  psq = 128*sqrt(p_c) (negative-poisoned for ignored pixels), qi = floor(psq)
  A = [valid | qi>=64 x20cls | fg-onehot x20]                     (41 cols)
  B = [(qi&63)>=32 x20 | ones | qsq>=32m (m=1..4) | fg ramps j=0..7] (33)
  qsq = psq at the label class; ramps uniform in e (phi resampled on host).
Bins are sqrt-spaced in p (edges p = (m/4)^2), which keeps the trapezoid
Phi integral accurate where b(t) varies fastest (small p).
Host tail (tiny): per-class 8-bin bg histogram = all-valid (2D suffix diff)
minus fg suffix counts (U cols); trapezoid Phi on the shared sqrt grid;
terms + include/count logic -> final scalar.  Valid/fg counts come from the
device (ones column), so the host never touches the big arrays.
"""

import math
import numpy as np
from contextlib import ExitStack

import concourse.bass as bass
import concourse.tile as tile
from concourse import bacc, mybir
from concourse.bass_utils import run_bass_kernel_spmd

F32 = mybir.dt.float32
BF16 = mybir.dt.bfloat16
I16 = mybir.dt.int16
ALU = mybir.AluOpType
ACTF = mybir.ActivationFunctionType
AXL = mybir.AxisListType

P = 128
C = 21
NCLS = 20
N_CORES = 8
NA = 41            # A cols: vld(1) + S1 (20) + fgm (20)
NB = 33            # B cols: T1 (20) + ones(1) + U (4) + R (8)
COL_FGM = 21
COL_ONES = 20
COL_U = 21
COL_R = 25
LNQ = math.log(128.0)   # psq = 128*sqrt(p)


def _ap(base, extra_off, dims):
    """Custom AP on a tile/dram AP: keep partition dim, replace free dims."""
    return bass.AP(tensor=base.tensor, offset=base.offset + extra_off,
                   ap=[list(base.ap[0])] + [list(d) for d in dims])


def _pin_act_tables():
    """Make the act-table-load pass assign every activation to the one set
    containing all funcs we use (natural_log_exp_and_others), instead of
    thrashing between the exp and ln sets (stateless first-match).  Set ids
    stay aligned with act_info.json because only set CONTENTS are filtered,
    never the order.  Returns a restore function."""
    import concourse.hw_specs as hw_specs
    orig = hw_specs.get_activation_tables
    mine = {ACTF.Exp, ACTF.Ln, ACTF.Square, ACTF.Relu}

    def patched(arch):
        t = orig(arch)
        out = {}
        for name, funcs in t.items():
            if name == "natural_log_exp_and_others":
                out[name] = funcs
            else:
                out[name] = funcs - mine
        return out

    hw_specs.get_activation_tables = patched
    import concourse.bacc as bacc_mod
    had = getattr(bacc_mod, "get_activation_tables", None)
    if had is not None:
        bacc_mod.get_activation_tables = patched

    def restore():
        hw_specs.get_activation_tables = orig
        if had is not None:
            bacc_mod.get_activation_tables = orig
    return restore


def build(ncols=2048, T=128):
    assert ncols % T == 0
    NT = ncols // T
    restore_tables = _pin_act_tables()
    nc = bacc.Bacc("TRN2", target_bir_lowering=False, debug=False,
                   enable_asserts=False, num_devices=N_CORES)
    lg_d = nc.dram_tensor("logits", [P, ncols, C], F32, kind="ExternalInput")
    lab_d = nc.dram_tensor("labels", [P, ncols], BF16, kind="ExternalInput")
    cst_d = nc.dram_tensor("consts", [P, 40], BF16, kind="ExternalInput")
    cstf_d = nc.dram_tensor("cstf", [P, 16], F32, kind="ExternalInput")
    out_d = nc.dram_tensor("out", [NA, NB], F32, kind="ExternalOutput")

    with tile.TileContext(nc) as tc, ExitStack() as ctx:
        singles = ctx.enter_context(tc.tile_pool(name="singles", bufs=1))
        pool = ctx.enter_context(tc.tile_pool(name="work", bufs=2))
        abpool = ctx.enter_context(tc.tile_pool(name="ab", bufs=3))
        psum = ctx.enter_context(
            tc.tile_pool(name="psum", bufs=1, space=bass.MemorySpace.PSUM))

        labs = singles.tile([P, ncols], BF16)
        nc.sync.dma_start(labs[:], lab_d.ap())
        cst = singles.tile([P, 40], BF16)
        nc.sync.dma_start(cst[:], cst_d.ap())
        cstf = singles.tile([P, 16], F32)
        nc.sync.dma_start(cstf[:], cstf_d.ap())

        ps = psum.tile([NA, NB], F32)
        lg_ap = lg_d.ap()
        labs_ap = labs[:]
        cst_ap = cst[:]
        # per-partition scalar bias APs: [LNQ]
        bias_ap = lambda k: _ap(cstf[:], k, [[1, 1]])

        for it in range(NT):
            t0 = it * T
            first = it == 0
            last = it == NT - 1

            lgt = pool.tile([P, T, C], F32, tag="lg")
            nc.sync.dma_start(lgt[:], _ap(lg_ap, t0 * C, [[C, T], [1, C]]))

            # Activation engine: ez = e^z, h = e^{z/2}
            ez = pool.tile([P, T, C], BF16, tag="ez")
            nc.scalar.activation(ez[:], lgt[:], ACTF.Exp)
            h = pool.tile([P, T, C], BF16, tag="h")
            nc.scalar.activation(h[:], lgt[:], ACTF.Exp, scale=0.5)

            # s = sum_c e^z (f32): in-place halving tree on ez, then reduce
            ez_ap = ez[:]
            nc.vector.tensor_tensor(
                _ap(ez_ap, 0, [[C, T], [1, 1]]),
                _ap(ez_ap, 0, [[C, T], [1, 1]]),
                _ap(ez_ap, 20, [[C, T], [1, 1]]), ALU.add)
            nc.vector.tensor_tensor(
                _ap(ez_ap, 0, [[C, T], [1, 10]]),
                _ap(ez_ap, 0, [[C, T], [1, 10]]),
                _ap(ez_ap, 10, [[C, T], [1, 10]]), ALU.add)
            nc.vector.tensor_tensor(
                _ap(ez_ap, 0, [[C, T], [1, 5]]),
                _ap(ez_ap, 0, [[C, T], [1, 5]]),
                _ap(ez_ap, 5, [[C, T], [1, 5]]), ALU.add)
            s = pool.tile([P, T], F32, tag="s")
            nc.vector.tensor_reduce(
                s[:], _ap(ez_ap, 0, [[C, T], [1, 5]]), axis=AXL.X, op=ALU.add)

            # Act: g = 128/sqrt(s) via exp(-0.5*ln s + ln 128)
            lns = pool.tile([P, T], F32, tag="lns")
            nc.scalar.activation(lns[:], s[:], ACTF.Ln)
            g = pool.tile([P, T], F32, tag="g")
            nc.scalar.activation(g[:], lns[:], ACTF.Exp, scale=-0.5, bias=bias_ap(0))

            # DVE: m2 = -2 on invalid pixels; g2 = (1+m2)*g duplicated pairs
            m2 = pool.tile([P, T], BF16, tag="m2")
            nc.vector.tensor_scalar(
                m2[:], _ap(labs_ap, t0, [[1, T]]), 0.0, -2.0,
                ALU.is_equal, ALU.mult)
            g2 = pool.tile([P, T, 2], BF16, tag="g2")
            nc.vector.scalar_tensor_tensor(
                _ap(g2[:], 0, [[2, T], [1, 2]]),
                _ap(m2[:], 0, [[1, T], [0, 2]]),
                1.0,
                _ap(g[:], 0, [[1, T], [0, 2]]),
                op0=ALU.add, op1=ALU.mult)

            # psq = h[1:21] * gp  (256*sqrt(p_c)), 2x via pairs
            psq = pool.tile([P, T, NCLS], BF16, tag="psq")
            nc.vector.tensor_tensor(
                _ap(psq[:], 0, [[NCLS, T], [2, 10], [1, 2]]),
                _ap(h[:], 1, [[C, T], [2, 10], [1, 2]]),
                _ap(g2[:], 0, [[2, T], [0, 10], [1, 2]]),
                ALU.mult)
            qi = pool.tile([P, T, NCLS], I16, tag="qi")
            nc.vector.tensor_scalar(qi[:], psq[:], -0.5, None, ALU.add)

            A = abpool.tile([P, T, NA], BF16, tag="A")
            B = abpool.tile([P, T, NB], BF16, tag="B")
            qi_ap = qi[:]

            # A col 0: valid
            nc.vector.tensor_scalar(
                _ap(A[:], 0, [[NA, T], [1, 1]]),
                _ap(labs_ap, t0, [[1, T], [0, 1]]),
                0.5, None, ALU.is_ge)
            # A cols 1..20: S1 = qi >= 64 (DVE 4x)
            nc.vector.tensor_scalar(
                _ap(A[:], 1, [[NA, T], [1, NCLS]]),
                qi_ap, 64, None, ALU.is_ge)
            # A cols 21..40: fg one-hot (DVE 2x via pairs)
            lab2 = pool.tile([P, T, 2], BF16, tag="lab2")
            nc.vector.tensor_copy(
                lab2[:], _ap(labs_ap, t0, [[1, T], [0, 2]]))
            nc.vector.tensor_tensor(
                _ap(A[:], COL_FGM, [[NA, T], [2, 10], [1, 2]]),
                _ap(lab2[:], 0, [[2, T], [0, 10], [1, 2]]),
                _ap(cst_ap, 0, [[0, T], [2, 10], [1, 2]]),
                ALU.is_equal)

            # B cols 0..19: T1 = (qi & 32) >= 1 (DVE 4x, two ops)
            mi = pool.tile([P, T, NCLS], I16, tag="mi")
            nc.vector.tensor_scalar(mi[:], qi_ap, 32, None, ALU.bitwise_and)
            nc.vector.tensor_scalar(
                _ap(B[:], 0, [[NB, T], [1, NCLS]]),
                mi[:], 1, None, ALU.is_ge)
            # B col 20: ones (Pool memset)
            nc.gpsimd.memset(_ap(B[:], COL_ONES, [[NB, T], [1, 1]]), 1.0)

            # qsq = psq at label (fg) via one-hot mult + halving tree reduce
            tmp = pool.tile([P, T, NCLS], BF16, tag="tmp")
            nc.vector.tensor_tensor(
                tmp[:],
                _ap(A[:], COL_FGM, [[NA, T], [1, NCLS]]),
                psq[:], ALU.mult)
            tmp_ap = tmp[:]
            nc.vector.tensor_tensor(
                _ap(tmp_ap, 0, [[NCLS, T], [1, 10]]),
                _ap(tmp_ap, 0, [[NCLS, T], [1, 10]]),
                _ap(tmp_ap, 10, [[NCLS, T], [1, 10]]), ALU.add)
            nc.vector.tensor_tensor(
                _ap(tmp_ap, 0, [[NCLS, T], [1, 5]]),
                _ap(tmp_ap, 0, [[NCLS, T], [1, 5]]),
                _ap(tmp_ap, 5, [[NCLS, T], [1, 5]]), ALU.add)
            qsq = pool.tile([P, T], F32, tag="qsq")
            nc.vector.tensor_reduce(
                qsq[:], _ap(tmp_ap, 0, [[NCLS, T], [1, 5]]),
                axis=AXL.X, op=ALU.add)
            # bf16 pairs of qsq
            qsq2 = pool.tile([P, T, 2], BF16, tag="qsq2")
            nc.vector.tensor_copy(
                qsq2[:], _ap(qsq[:], 0, [[1, T], [0, 2]]))
            # B cols 21..24: U_m = qsq >= 32m (DVE 2x via pairs)
            nc.vector.tensor_tensor(
                _ap(B[:], COL_U, [[NB, T], [2, 2], [1, 2]]),
                _ap(qsq2[:], 0, [[2, T], [0, 2], [1, 2]]),
                _ap(cst_ap, NCLS, [[0, T], [2, 2], [1, 2]]),
                ALU.is_ge)

            # fg ramps uniform in e: x8 = 8*e = (256.001 - (qsq/8)^2)/32
            qfg = pool.tile([P, T], F32, tag="qfg")
            nc.scalar.activation(qfg[:], qsq[:], ACTF.Square, scale=1.0 / 8.0)
            x8 = pool.tile([P, T, 2], BF16, tag="x8")
            nc.vector.tensor_scalar(
                _ap(x8[:], 0, [[2, T], [1, 2]]),
                _ap(qfg[:], 0, [[1, T], [0, 2]]),
                -1.0 / 32.0, 8.00003, ALU.mult, ALU.add)
            u = pool.tile([P, T, 8], BF16, tag="u")
            nc.vector.tensor_tensor(
                _ap(u[:], 0, [[8, T], [2, 4], [1, 2]]),
                _ap(x8[:], 0, [[2, T], [0, 4], [1, 2]]),
                _ap(cst_ap, NCLS + 8, [[0, T], [2, 4], [1, 2]]),
                ALU.subtract)
            nc.vector.tensor_scalar(
                _ap(B[:], COL_R, [[NB, T], [1, 8]]),
                u[:], 1.0, 0.0, ALU.min, ALU.max)

            A_ap = A[:]
            B_ap = B[:]
            for tt_ in range(T):
                nc.tensor.matmul(
                    ps[:],
                    _ap(A_ap, tt_ * NA, [[1, NA]]),
                    _ap(B_ap, tt_ * NB, [[1, NB]]),
                    start=(first and tt_ == 0),
                    stop=(last and tt_ == T - 1))

        outs = singles.tile([NA, NB], F32)
        nc.vector.tensor_copy(outs[:], ps[:])
        nc.sync.dma_start(out_d.ap(), outs[:])

    nc.compile()
    restore_tables()
    return nc


def make_consts():
    row = np.zeros(40, np.float32)
    row[0:20] = np.arange(1, 21)           # fgm class ids
    row[20:24] = 32.0 * np.arange(1, 5)    # U edges (qsq units)
    row[28:36] = np.arange(8)              # ramp nodes (x8 units)
    import ml_dtypes
    cb = np.ascontiguousarray(
        np.broadcast_to(row.astype(ml_dtypes.bfloat16), (P, 40)))
    rf = np.zeros(16, np.float32)
    rf[0] = LNQ
    cf = np.ascontiguousarray(np.broadcast_to(rf, (P, 16)))
    return cb, cf


def host_tail(out):
    """out[NA, NB] f32 -> (terms[20], V) in float64."""
    o = out.astype(np.float64)
    vld_row = o[0]
    S_rows = o[1:21]                       # S1, (20, NB)
    F_rows = o[21:41]                      # fgm rows
    V = vld_row[COL_ONES]
    G = F_rows[:, COL_ONES]

    # 2D suffix counts (a in {0,1} x b in {0,1}) -> 4-bin counts -> bg suffix
    ar = np.arange(NCLS)
    Cs = np.zeros((NCLS, 3, 2))
    Cs[:, 0, 0] = V
    Cs[:, 0, 1] = vld_row[:NCLS]
    Cs[:, 1, 0] = S_rows[:, COL_ONES]
    Cs[:, 1, 1] = S_rows[ar, ar]
    D = Cs[:, :2, :] - Cs[:, 1:3, :]
    hcell = D.copy()
    hcell[:, :, 0] -= hcell[:, :, 1]
    hflat = hcell.reshape(NCLS, 4)
    bcnt = np.concatenate(
        [np.cumsum(hflat[:, ::-1], axis=1)[:, ::-1],
         np.zeros((NCLS, 1))], axis=1)                  # (20, 5) suffix m=0..4
    F = np.concatenate([G[:, None], F_rows[:, COL_U:COL_U + 4]], axis=1)
    bg = np.maximum(bcnt - F, 0.0)
    r = 1.0 / np.maximum(G[:, None] + bg, 1.0)
    edges = (np.arange(5) / 4.0) ** 2
    dphi = (r[:, :-1] + r[:, 1:]) * 0.5 * np.diff(edges)[None, :]
    phiedges = np.concatenate(
        [np.zeros((NCLS, 1)), np.cumsum(dphi, axis=1)], axis=1)
    phi1 = phiedges[:, -1]
    # resample phi (piecewise-linear in e) onto the uniform 8-ramp grid
    enodes = np.arange(9) / 8.0
    phin = np.empty((NCLS, 9))
    for c in range(NCLS):
        phin[c] = np.interp(enodes, edges, phiedges[c])
    fg_term = (np.diff(phin, axis=1) * F_rows[:, COL_R:COL_R + 8]).sum(axis=1)
    terms = fg_term + 1.0 - G * phi1
    return terms, V


_NC_CACHE = {}


def _get_nc():
    if "nc" not in _NC_CACHE:
        _NC_CACHE["nc"] = build()
    return _NC_CACHE["nc"]


def _get_exec():
    """Cached jitted SPMD executor taking global (concat) arrays directly,
    avoiding run_bass_via_pjrt's per-call 176MB concat + jit retrace."""
    if "exec" in _NC_CACHE:
        return _NC_CACHE["exec"]
    import jax
    from jax.sharding import Mesh, PartitionSpec
    from concourse.bass2jax import (
        _bass_exec_p, install_neuronx_cc_hook, partition_id_tensor)

    nc = _get_nc()
    install_neuronx_cc_hook()
    partition_name = (nc.partition_id_tensor.name
                      if nc.partition_id_tensor else None)

    in_names, out_names, out_avals, zero_outs = [], [], [], []
    for alloc in nc.m.functions[0].allocations:
        if not isinstance(alloc, mybir.MemoryLocationSet):
            continue
        name = alloc.memorylocations[0].name
        if alloc.kind == "ExternalInput":
            if name != partition_name:
                in_names.append(name)
        elif alloc.kind == "ExternalOutput":
            shape = tuple(alloc.tensor_shape)
            dtype = mybir.dt.np(alloc.dtype)
            out_names.append(name)
            out_avals.append(jax.core.ShapedArray(shape, dtype))
            zero_outs.append(
                np.zeros((N_CORES * shape[0], *shape[1:]), dtype))
    n_params = len(in_names)
    all_names = in_names + out_names
    if partition_name is not None:
        all_names = all_names + [partition_name]
    donate = tuple(range(n_params, n_params + len(out_names)))

    def _body(*args):
        operands = list(args)
        if partition_name is not None:
            operands.append(partition_id_tensor())
        outs = _bass_exec_p.bind(
            *operands,
            out_avals=tuple(out_avals),
            in_names=tuple(all_names),
            out_names=tuple(out_names),
            lowering_input_output_aliases=(),
            sim_require_finite=True,
            sim_require_nnan=True,
            nc=nc,
        )
        return tuple(outs)

    try:
        from jax import shard_map
    except ImportError:
        from jax.experimental.shard_map import shard_map
    devices = jax.devices()[:N_CORES]
    assert len(devices) == N_CORES, (
        f"need {N_CORES} devices, have {len(jax.devices())}")
    mesh = Mesh(np.asarray(devices), ("core",))
    nio = n_params + len(out_names)
    sharded = jax.jit(
        shard_map(_body, mesh=mesh,
                  in_specs=(PartitionSpec("core"),) * nio,
                  out_specs=(PartitionSpec("core"),) * len(out_names),
                  check_vma=False),
        donate_argnums=donate, keep_unused=True)
    _NC_CACHE["exec"] = (sharded, in_names, out_names, out_avals, zero_outs)
    return _NC_CACHE["exec"]


def kernel(logits, labels):
    import os
    import ml_dtypes
    B, N, Cin = logits.shape
    assert (B, Cin) == (N_CORES, C) and N % P == 0
    ncols = N // P

    logits_g = np.ascontiguousarray(logits, dtype=np.float32) \
        .reshape(B * P, ncols, C)
    labf_g = labels.reshape(B * P, ncols).astype(ml_dtypes.bfloat16)
    consts, constsf = make_consts()
    consts_g = np.ascontiguousarray(
        np.broadcast_to(consts[None, 0], (B * P, consts.shape[1])))
    constsf_g = np.ascontiguousarray(
        np.broadcast_to(constsf[None, 0], (B * P, constsf.shape[1])))

    if os.environ.get("BASS_TRACE"):
        # slow traced path through run_bass_kernel_spmd for profiling
        nc = _get_nc()
        in_maps = [
            {"logits": logits_g[b * P:(b + 1) * P],
             "labels": labf_g[b * P:(b + 1) * P],
             "consts": consts, "cstf": constsf}
            for b in range(B)
        ]
        res = run_bass_kernel_spmd(nc, in_maps, core_ids=list(range(N_CORES)))
        _NC_CACHE["last_results"] = res
        outs = [res.results[b]["out"] for b in range(B)]
    else:
        sharded, in_names, out_names, out_avals, zero_outs = _get_exec()
        arrs = {"logits": logits_g, "labels": labf_g,
                "consts": consts_g, "cstf": constsf_g}
        out_arrs = sharded(*[arrs[n] for n in in_names],
                           *[z.copy() for z in zero_outs])
        og = np.asarray(out_arrs[out_names.index("out")])
        outs = [og.reshape(N_CORES, NA, NB)[b] for b in range(B)]

    total = 0.0
    n_included = 0
    for b in range(B):
        terms, V = host_tail(outs[b])
        if V >= 2:
            total += terms.sum()
            n_included += 1
    count = max(n_included * (C - 1), 1)
    return np.float32(total / count)


# revision 33
# speedup vs baseline: 3.0879x; 1.0062x over previous
"""Lovasz-Softmax loss on Trainium2 (Bass/Tile), 8-core data-parallel over batch.

Math: for each (batch, class c>=1) the Lovasz term equals
    term = sum_{fg pixels i} Phi(e_i) + 1 - G * Phi(1)
where e_i = 1 - p_c(i), G = #fg pixels, Phi(x) = int_0^x dt / (G + b(t)),
and b(t) = #background-valid pixels with p_c > t.  (Abel summation; exact.)

Device work per core (1 batch of 262144 pixels x 21 classes), per 128-pixel
group one 41x34 bf16 matmul accumulated in PSUM:
  psq = 128*sqrt(p_c) (negative-poisoned for ignored pixels), qi = floor(psq)
  A = [valid | qi>=64 x20cls | fg-onehot x20]                     (41 cols)
  B = [(qi&63)>=32 x20 | qsq>=32m (m=1..4) | ones/ramps j=0..7/dead] (34)
  qsq = psq at the label class; ramps uniform in e (phi resampled on host).
Bins are sqrt-spaced in p (edges p = (m/4)^2), which keeps the trapezoid
Phi integral accurate where b(t) varies fastest (small p).
Host tail (tiny): per-class 8-bin bg histogram = all-valid (2D suffix diff)
minus fg suffix counts (U cols); trapezoid Phi on the shared sqrt grid;
terms + include/count logic -> final scalar.  Valid/fg counts come from the
device (ones column), so the host never touches the big arrays.
"""

import math
import numpy as np
from contextlib import ExitStack

import concourse.bass as bass
import concourse.tile as tile
from concourse import bacc, mybir
from concourse.bass_utils import run_bass_kernel_spmd

F32 = mybir.dt.float32
BF16 = mybir.dt.bfloat16
I16 = mybir.dt.int16
ALU = mybir.AluOpType
ACTF = mybir.ActivationFunctionType
AXL = mybir.AxisListType

P = 128
C = 21
NCLS = 20
N_CORES = 8
NA = 41            # A cols: vld(1) + S1 (20) + fgm (20)
NB = 34            # B: T1 (20) + U (4) + R: ones/ramps j=0..7/dead (10)
COL_FGM = 21
COL_U = 20
COL_R = 24         # R node vector [-1000, 0..7, +1000]; col 24 == ones
COL_ONES = 24
LNQ = math.log(128.0)   # psq = 128*sqrt(p)


def _ap(base, extra_off, dims):
    """Custom AP on a tile/dram AP: keep partition dim, replace free dims."""
    return bass.AP(tensor=base.tensor, offset=base.offset + extra_off,
                   ap=[list(base.ap[0])] + [list(d) for d in dims])


def _pin_act_tables():
    """Make the act-table-load pass assign every activation to the one set
    containing all funcs we use (natural_log_exp_and_others), instead of
    thrashing between the exp and ln sets (stateless first-match).  Set ids
    stay aligned with act_info.json because only set CONTENTS are filtered,
    never the order.  Returns a restore function."""
    import concourse.hw_specs as hw_specs
    orig = hw_specs.get_activation_tables
    mine = {ACTF.Exp, ACTF.Ln, ACTF.Square, ACTF.Relu}

    def patched(arch):
        t = orig(arch)
        out = {}
        for name, funcs in t.items():
            if name == "natural_log_exp_and_others":
                out[name] = funcs
            else:
                out[name] = funcs - mine
        return out

    hw_specs.get_activation_tables = patched
    import concourse.bacc as bacc_mod
    had = getattr(bacc_mod, "get_activation_tables", None)
    if had is not None:
        bacc_mod.get_activation_tables = patched

    def restore():
        hw_specs.get_activation_tables = orig
        if had is not None:
            bacc_mod.get_activation_tables = orig
    return restore


def build(ncols=2048, T=256):
    assert ncols % T == 0
    NT = ncols // T
    restore_tables = _pin_act_tables()
    nc = bacc.Bacc("TRN2", target_bir_lowering=False, debug=False,
                   enable_asserts=False, num_devices=N_CORES)
    lg_d = nc.dram_tensor("logits", [P, ncols, C], F32, kind="ExternalInput")
    lab_d = nc.dram_tensor("labels", [P, ncols], BF16, kind="ExternalInput")
    cst_d = nc.dram_tensor("consts", [P, 40], BF16, kind="ExternalInput")
    cstf_d = nc.dram_tensor("cstf", [P, 16], F32, kind="ExternalInput")
    out_d = nc.dram_tensor("out", [NA, NB], F32, kind="ExternalOutput")

    with tile.TileContext(nc) as tc, ExitStack() as ctx:
        singles = ctx.enter_context(tc.tile_pool(name="singles", bufs=1))
        pool = ctx.enter_context(tc.tile_pool(name="work", bufs=2))
        abpool = ctx.enter_context(tc.tile_pool(name="ab", bufs=2))
        psum = ctx.enter_context(
            tc.tile_pool(name="psum", bufs=1, space=bass.MemorySpace.PSUM))

        cst = singles.tile([P, 40], BF16)
        nc.sync.dma_start(cst[:], cst_d.ap())
        cstf = singles.tile([P, 16], F32)
        nc.sync.dma_start(cstf[:], cstf_d.ap())

        ps = psum.tile([NA, NB], F32)
        lg_ap = lg_d.ap()
        lab_ap = lab_d.ap()
        cst_ap = cst[:]
        # per-partition scalar bias APs: [LNQ]
        bias_ap = lambda k: _ap(cstf[:], k, [[1, 1]])

        for it in range(NT):
            t0 = it * T
            first = it == 0
            last = it == NT - 1

            labs = pool.tile([P, T], BF16, tag="labs")
            nc.sync.dma_start(labs[:], _ap(lab_ap, t0, [[1, T]]))
            labs_ap = labs[:]

            # Activation engine: ez = e^z, h = e^{z/2} (half-chunk granularity
            # so the f32 logits staging tile stays small)
            TH = T // 2
            ez = pool.tile([P, T, C], BF16, tag="ez")
            h = pool.tile([P, T, C], BF16, tag="h")
            for hf in range(2):
                lgt = pool.tile([P, TH, C], F32, tag="lg")
                nc.sync.dma_start(
                    lgt[:], _ap(lg_ap, (t0 + hf * TH) * C, [[C, TH], [1, C]]))
                nc.scalar.activation(
                    _ap(ez[:], hf * TH * C, [[C, TH], [1, C]]),
                    lgt[:], ACTF.Exp)
                nc.scalar.activation(
                    _ap(h[:], hf * TH * C, [[C, TH], [1, C]]),
                    lgt[:], ACTF.Exp, scale=0.5)

            # s = sum_c e^z: in-place halving tree on ez, then reduce (DVE)
            ez_ap = ez[:]
            nc.vector.tensor_tensor(
                _ap(ez_ap, 0, [[C, T], [1, 1]]),
                _ap(ez_ap, 0, [[C, T], [1, 1]]),
                _ap(ez_ap, 20, [[C, T], [1, 1]]), ALU.add)
            nc.vector.tensor_tensor(
                _ap(ez_ap, 0, [[C, T], [1, 10]]),
                _ap(ez_ap, 0, [[C, T], [1, 10]]),
                _ap(ez_ap, 10, [[C, T], [1, 10]]), ALU.add)
            nc.vector.tensor_tensor(
                _ap(ez_ap, 0, [[C, T], [1, 5]]),
                _ap(ez_ap, 0, [[C, T], [1, 5]]),
                _ap(ez_ap, 5, [[C, T], [1, 5]]), ALU.add)
            s = pool.tile([P, T], F32, tag="s")
            nc.vector.tensor_reduce(
                s[:], _ap(ez_ap, 0, [[C, T], [1, 5]]), axis=AXL.X, op=ALU.add)

            # Act: g = 128/sqrt(s) via exp(-0.5*ln s + ln 128)
            lns = pool.tile([P, T], F32, tag="lns")
            nc.scalar.activation(lns[:], s[:], ACTF.Ln)
            g = pool.tile([P, T], F32, tag="g")
            nc.scalar.activation(g[:], lns[:], ACTF.Exp, scale=-0.5, bias=bias_ap(0))

            # DVE: m2 = -2 on invalid pixels; g2 = (1+m2)*g duplicated pairs
            m2 = pool.tile([P, T], BF16, tag="m2")
            nc.vector.tensor_scalar(
                m2[:], _ap(labs_ap, 0, [[1, T]]), 0.0, -2.0,
                ALU.is_equal, ALU.mult)
            g2 = pool.tile([P, T, 2], BF16, tag="g2")
            nc.vector.scalar_tensor_tensor(
                _ap(g2[:], 0, [[2, T], [1, 2]]),
                _ap(m2[:], 0, [[1, T], [0, 2]]),
                1.0,
                _ap(g[:], 0, [[1, T], [0, 2]]),
                op0=ALU.add, op1=ALU.mult)

            # psq = h[1:21] * gp  (256*sqrt(p_c)), 2x via pairs
            psq = pool.tile([P, T, NCLS], BF16, tag="psq")
            nc.vector.tensor_tensor(
                _ap(psq[:], 0, [[NCLS, T], [2, 10], [1, 2]]),
                _ap(h[:], 1, [[C, T], [2, 10], [1, 2]]),
                _ap(g2[:], 0, [[2, T], [0, 10], [1, 2]]),
                ALU.mult)
            qi = pool.tile([P, T, NCLS], I16, tag="qi")
            nc.vector.tensor_scalar(qi[:], psq[:], -0.5, None, ALU.add)

            A = abpool.tile([P, T, NA], BF16, tag="A")
            B = abpool.tile([P, T, NB], BF16, tag="B")
            qi_ap = qi[:]

            # A col 0: valid
            nc.vector.tensor_scalar(
                _ap(A[:], 0, [[NA, T], [1, 1]]),
                _ap(labs_ap, 0, [[1, T], [0, 1]]),
                0.5, None, ALU.is_ge)
            # A cols 1..20: S1 = qi >= 64 (DVE 4x)
            nc.vector.tensor_scalar(
                _ap(A[:], 1, [[NA, T], [1, NCLS]]),
                qi_ap, 64, None, ALU.is_ge)
            # A cols 21..40: fg one-hot (DVE 2x via pairs)
            lab2 = pool.tile([P, T, 2], BF16, tag="lab2")
            nc.vector.tensor_copy(
                lab2[:], _ap(labs_ap, 0, [[1, T], [0, 2]]))
            nc.vector.tensor_tensor(
                _ap(A[:], COL_FGM, [[NA, T], [2, 10], [1, 2]]),
                _ap(lab2[:], 0, [[2, T], [0, 10], [1, 2]]),
                _ap(cst_ap, 0, [[0, T], [2, 10], [1, 2]]),
                ALU.is_equal)

            # B cols 0..19: T1 = (qi & 32) >= 1 (DVE 4x, two ops;
            # the AND overwrites qi in place -- S1 reads it first)
            nc.vector.tensor_scalar(qi_ap, qi_ap, 32, None, ALU.bitwise_and)
            nc.vector.tensor_scalar(
                _ap(B[:], 0, [[NB, T], [1, NCLS]]),
                qi_ap, 1, None, ALU.is_ge)
            # qsq = psq at label (fg) via one-hot mult + halving tree reduce
            # (the mult overwrites psq in place -- qi/S1 consumed it already)
            nc.vector.tensor_tensor(
                psq[:],
                _ap(A[:], COL_FGM, [[NA, T], [1, NCLS]]),
                psq[:], ALU.mult)
            tmp_ap = psq[:]
            nc.vector.tensor_tensor(
                _ap(tmp_ap, 0, [[NCLS, T], [1, 10]]),
                _ap(tmp_ap, 0, [[NCLS, T], [1, 10]]),
                _ap(tmp_ap, 10, [[NCLS, T], [1, 10]]), ALU.add)
            nc.vector.tensor_tensor(
                _ap(tmp_ap, 0, [[NCLS, T], [1, 5]]),
                _ap(tmp_ap, 0, [[NCLS, T], [1, 5]]),
                _ap(tmp_ap, 5, [[NCLS, T], [1, 5]]), ALU.add)
            qsq = pool.tile([P, T], F32, tag="qsq")
            nc.vector.tensor_reduce(
                qsq[:], _ap(tmp_ap, 0, [[NCLS, T], [1, 5]]),
                axis=AXL.X, op=ALU.add)
            # bf16 pairs of qsq
            qsq2 = pool.tile([P, T, 2], BF16, tag="qsq2")
            nc.vector.tensor_copy(
                qsq2[:], _ap(qsq[:], 0, [[1, T], [0, 2]]))
            # B cols 20..23: U_m = qsq >= 32m (DVE 2x via pairs)
            nc.vector.tensor_tensor(
                _ap(B[:], COL_U, [[NB, T], [2, 2], [1, 2]]),
                _ap(qsq2[:], 0, [[2, T], [0, 2], [1, 2]]),
                _ap(cst_ap, NCLS, [[0, T], [2, 2], [1, 2]]),
                ALU.is_ge)

            # fg ramps uniform in e: x8 = 8*e = (256.001 - (qsq/8)^2)/32
            qfg = pool.tile([P, T], F32, tag="qfg")
            nc.scalar.activation(qfg[:], qsq[:], ACTF.Square, scale=1.0 / 8.0)
            x8 = pool.tile([P, T, 2], BF16, tag="x8")
            nc.vector.tensor_scalar(
                _ap(x8[:], 0, [[2, T], [1, 2]]),
                _ap(qfg[:], 0, [[1, T], [0, 2]]),
                -1.0 / 32.0, 8.00003, ALU.mult, ALU.add)
            nc.vector.tensor_tensor(
                _ap(B[:], COL_R, [[NB, T], [2, 5], [1, 2]]),
                _ap(x8[:], 0, [[2, T], [0, 5], [1, 2]]),
                _ap(cst_ap, NCLS + 8, [[0, T], [2, 5], [1, 2]]),
                ALU.subtract)
            nc.vector.tensor_scalar(
                _ap(B[:], COL_R, [[NB, T], [1, 10]]),
                _ap(B[:], COL_R, [[NB, T], [1, 10]]),
                1.0, 0.0, ALU.min, ALU.max)

            A_ap = A[:]
            B_ap = B[:]
            for tt_ in range(T):
                nc.tensor.matmul(
                    ps[:],
                    _ap(A_ap, tt_ * NA, [[1, NA]]),
                    _ap(B_ap, tt_ * NB, [[1, NB]]),
                    start=(first and tt_ == 0),
                    stop=(last and tt_ == T - 1))

        outs = singles.tile([NA, NB], F32)
        nc.vector.tensor_copy(outs[:], ps[:])
        nc.sync.dma_start(out_d.ap(), outs[:])

    nc.compile()
    restore_tables()
    return nc


def make_consts():
    row = np.zeros(40, np.float32)
    row[0:20] = np.arange(1, 21)           # fgm class ids
    row[20:24] = 32.0 * np.arange(1, 5)    # U edges (qsq units)
    row[28] = -1000.0                      # ones column (clips to 1)
    row[29:37] = np.arange(8)              # ramp nodes (x8 units)
    row[37] = 1000.0                       # dead column (clips to 0)
    import ml_dtypes
    cb = np.ascontiguousarray(
        np.broadcast_to(row.astype(ml_dtypes.bfloat16), (P, 40)))
    rf = np.zeros(16, np.float32)
    rf[0] = LNQ
    cf = np.ascontiguousarray(np.broadcast_to(rf, (P, 16)))
    return cb, cf


def host_tail(out):
    """out[NA, NB] f32 -> (terms[20], V) in float64."""
    o = out.astype(np.float64)
    vld_row = o[0]
    S_rows = o[1:21]                       # S1, (20, NB)
    F_rows = o[21:41]                      # fgm rows
    V = vld_row[COL_ONES]
    G = F_rows[:, COL_ONES]

    # 2D suffix counts (a in {0,1} x b in {0,1}) -> 4-bin counts -> bg suffix
    ar = np.arange(NCLS)
    Cs = np.zeros((NCLS, 3, 2))
    Cs[:, 0, 0] = V
    Cs[:, 0, 1] = vld_row[:NCLS]
    Cs[:, 1, 0] = S_rows[:, COL_ONES]
    Cs[:, 1, 1] = S_rows[ar, ar]
    D = Cs[:, :2, :] - Cs[:, 1:3, :]
    hcell = D.copy()
    hcell[:, :, 0] -= hcell[:, :, 1]
    hflat = hcell.reshape(NCLS, 4)
    bcnt = np.concatenate(
        [np.cumsum(hflat[:, ::-1], axis=1)[:, ::-1],
         np.zeros((NCLS, 1))], axis=1)                  # (20, 5) suffix m=0..4
    F = np.concatenate([G[:, None], F_rows[:, COL_U:COL_U + 4]], axis=1)
    bg = np.maximum(bcnt - F, 0.0)
    r = 1.0 / np.maximum(G[:, None] + bg, 1.0)
    edges = (np.arange(5) / 4.0) ** 2
    dphi = (r[:, :-1] + r[:, 1:]) * 0.5 * np.diff(edges)[None, :]
    phiedges = np.concatenate(
        [np.zeros((NCLS, 1)), np.cumsum(dphi, axis=1)], axis=1)
    phi1 = phiedges[:, -1]
    # resample phi (piecewise-linear in e) onto the uniform 8-ramp grid
    enodes = np.arange(9) / 8.0
    phin = np.empty((NCLS, 9))
    for c in range(NCLS):
        phin[c] = np.interp(enodes, edges, phiedges[c])
    fg_term = (np.diff(phin, axis=1)
               * F_rows[:, COL_R + 1:COL_R + 9]).sum(axis=1)
    terms = fg_term + 1.0 - G * phi1
    return terms, V


_NC_CACHE = {}


def _get_nc():
    if "nc" not in _NC_CACHE:
        _NC_CACHE["nc"] = build()
    return _NC_CACHE["nc"]


def _get_exec():
    """Cached jitted SPMD executor taking global (concat) arrays directly,
    avoiding run_bass_via_pjrt's per-call 176MB concat + jit retrace."""
    if "exec" in _NC_CACHE:
        return _NC_CACHE["exec"]
    import jax
    from jax.sharding import Mesh, PartitionSpec
    from concourse.bass2jax import (
        _bass_exec_p, install_neuronx_cc_hook, partition_id_tensor)

    nc = _get_nc()
    install_neuronx_cc_hook()
    partition_name = (nc.partition_id_tensor.name
                      if nc.partition_id_tensor else None)

    in_names, out_names, out_avals, zero_outs = [], [], [], []
    for alloc in nc.m.functions[0].allocations:
        if not isinstance(alloc, mybir.MemoryLocationSet):
            continue
        name = alloc.memorylocations[0].name
        if alloc.kind == "ExternalInput":
            if name != partition_name:
                in_names.append(name)
        elif alloc.kind == "ExternalOutput":
            shape = tuple(alloc.tensor_shape)
            dtype = mybir.dt.np(alloc.dtype)
            out_names.append(name)
            out_avals.append(jax.core.ShapedArray(shape, dtype))
            zero_outs.append(
                np.zeros((N_CORES * shape[0], *shape[1:]), dtype))
    n_params = len(in_names)
    all_names = in_names + out_names
    if partition_name is not None:
        all_names = all_names + [partition_name]
    donate = tuple(range(n_params, n_params + len(out_names)))

    def _body(*args):
        operands = list(args)
        if partition_name is not None:
            operands.append(partition_id_tensor())
        outs = _bass_exec_p.bind(
            *operands,
            out_avals=tuple(out_avals),
            in_names=tuple(all_names),
            out_names=tuple(out_names),
            lowering_input_output_aliases=(),
            sim_require_finite=True,
            sim_require_nnan=True,
            nc=nc,
        )
        return tuple(outs)

    try:
        from jax import shard_map
    except ImportError:
        from jax.experimental.shard_map import shard_map
    devices = jax.devices()[:N_CORES]
    assert len(devices) == N_CORES, (
        f"need {N_CORES} devices, have {len(jax.devices())}")
    mesh = Mesh(np.asarray(devices), ("core",))
    nio = n_params + len(out_names)
    sharded = jax.jit(
        shard_map(_body, mesh=mesh,
                  in_specs=(PartitionSpec("core"),) * nio,
                  out_specs=(PartitionSpec("core"),) * len(out_names),
                  check_vma=False),
        donate_argnums=donate, keep_unused=True)
    _NC_CACHE["exec"] = (sharded, in_names, out_names, out_avals, zero_outs)
    return _NC_CACHE["exec"]


def kernel(logits, labels):
    import os
    import ml_dtypes
    B, N, Cin = logits.shape
    assert (B, Cin) == (N_CORES, C) and N % P == 0
    ncols = N // P

    logits_g = np.ascontiguousarray(logits, dtype=np.float32) \
        .reshape(B * P, ncols, C)
    labf_g = labels.reshape(B * P, ncols).astype(ml_dtypes.bfloat16)
    consts, constsf = make_consts()
    consts_g = np.ascontiguousarray(
        np.broadcast_to(consts[None, 0], (B * P, consts.shape[1])))
    constsf_g = np.ascontiguousarray(
        np.broadcast_to(constsf[None, 0], (B * P, constsf.shape[1])))

    if os.environ.get("BASS_TRACE"):
        # slow traced path through run_bass_kernel_spmd for profiling
        nc = _get_nc()
        in_maps = [
            {"logits": logits_g[b * P:(b + 1) * P],
             "labels": labf_g[b * P:(b + 1) * P],
             "consts": consts, "cstf": constsf}
            for b in range(B)
        ]
        res = run_bass_kernel_spmd(nc, in_maps, core_ids=list(range(N_CORES)))
        _NC_CACHE["last_results"] = res
        outs = [res.results[b]["out"] for b in range(B)]
    else:
        sharded, in_names, out_names, out_avals, zero_outs = _get_exec()
        arrs = {"logits": logits_g, "labels": labf_g,
                "consts": consts_g, "cstf": constsf_g}
        out_arrs = sharded(*[arrs[n] for n in in_names],
                           *[z.copy() for z in zero_outs])
        og = np.asarray(out_arrs[out_names.index("out")])
        outs = [og.reshape(N_CORES, NA, NB)[b] for b in range(B)]

    total = 0.0
    n_included = 0
    for b in range(B):
        terms, V = host_tail(outs[b])
        if V >= 2:
            total += terms.sum()
            n_included += 1
    count = max(n_included * (C - 1), 1)
    return np.float32(total / count)
